# revision 12
# baseline (speedup 1.0000x reference)
"""Beam-search step (CaptionModel) on 8 Trainium2 NeuronCores.

Distribution strategy (vocab-sharded, per the sharding hint):
  * logprobsf [128, 128000] f32 is sharded along the vocab dim: each of the
    8 cores streams a [128, 16000] shard through SBUF (tile-major
    contiguous DMA), and on the DVE reduces each tile with a fold-max
    cascade (window of 8) before extracting the per-tile top-8
    window-maxima + positions with the Max8 / MaxIndex instructions.
    That's ~1.17 DVE element-passes per input element, roughly matching
    the per-core HBM DMA floor.
  * Key reduction: the reference's per-row top-k followed by the global
    top-k over `beam_logprob_sum[q] + ys` equals one global top-128 of
    `beam_logprob_sum[q] + logprobsf[q, v]`, so the host only merges the
    8 cores' candidate lists (48 window-maxima per row per core, ~100 KB
    total) and rescans the ~128 winning 8-element windows (~1 KB reads).
    An exact threshold check (any chunk whose 8th-largest candidate could
    still hide a winner is rescanned) keeps the result exact, always.
  * A second tiny NEFF gathers the recurrent state along the beam dim with
    one indirect DMA per core (128 row descriptors over a 128-wide hidden
    slice of both states; data-parallel over the hidden dim).

The harness calls kernel(**inputs) with full inputs; both Bass programs are
built/compiled once and cached at module level.
"""

import numpy as np

# Problem geometry (hardcoded per spec).
T = 128          # seq length
B = 128          # beam rows
V = 128000       # vocab
L = 2            # layers
H = 1024         # hidden
NCORES = 8
VSH = V // NCORES                  # 16000 vocab cols per core
WIN = 8                            # fold window
TILES = (1600, 4000, 4000, 4000, 1600, 800)   # sums to VSH
assert sum(TILES) == VSH and all(w % WIN == 0 for w in TILES)
NCHUNK = len(TILES)                # one max8-chunk per tile
CAND = NCHUNK * 8                  # 48 candidates per row per core

_NC_A = None
_NC_B = None
_TRACE = False
_LAST_RESULTS = {}


def _build_a():
    """Program A: per-core fold-max candidate scan of the logit shard."""
    global _NC_A
    if _NC_A is not None:
        return _NC_A
    from concourse import bacc, tile, mybir
    dt = mybir.dt
    MAX = mybir.AluOpType.max

    nc = bacc.Bacc("TRN2", target_bir_lowering=False, debug=False,
                   num_devices=NCORES)
    lps = [nc.dram_tensor(f"lp{i}", [B, W], dt.float32, kind="ExternalInput")
           for i, W in enumerate(TILES)]
    bias = nc.dram_tensor("bias", [B, 1], dt.float32, kind="ExternalInput")
    cvals = nc.dram_tensor("cvals", [B, CAND], dt.bfloat16,
                           kind="ExternalOutput")
    cidx = nc.dram_tensor("cidx", [B, CAND], dt.uint16, kind="ExternalOutput")

    with tile.TileContext(nc) as tc:
        with tc.tile_pool(name="io", bufs=3) as iop, \
             tc.tile_pool(name="fold", bufs=2) as fp, \
             tc.tile_pool(name="cand", bufs=1) as cp:
            vals = cp.tile([B, CAND], dt.bfloat16)
            idxs = cp.tile([B, CAND], dt.uint16)
            bias_t = cp.tile([B, 1], dt.float32)
            # Keep the input (sync/SP) HWDGE ring exclusively for the big
            # tile loads -- it drains FIFO per issuing engine, so anything
            # else on it stalls later tiles.  Small transfers go on the
            # scalar (ACT) ring.
            nc.scalar.dma_start(bias_t[:], bias[:])
            for ti, W in enumerate(TILES):
                t = iop.tile([B, max(TILES)], dt.float32, tag="lp")
                nc.sync.dma_start(t[:, 0:W], lps[ti][:])
                if ti == NCHUNK - 1:
                    # EOS suppression: bias is -1000 on the core owning the
                    # last vocab column, 0 elsewhere (same program on all
                    # cores).
                    nc.vector.tensor_add(out=t[:, W - 1:W],
                                         in0=t[:, W - 1:W], in1=bias_t[:])
                # ACT casts to bf16 so the DVE folds run in 2x mode; the
                # host rescans winner windows against the exact f32 input,
                # so the bf16 rounding only widens the rescan margin.
                bf = fp.tile([B, max(TILES)], dt.bfloat16, tag="bf")
                nc.scalar.copy(bf[:, 0:W], t[:, 0:W])
                # fold-max cascade: W -> W/2 -> W/4 -> W/8
                h1 = fp.tile([B, max(TILES) // 2], dt.bfloat16, tag="h1")
                h2 = fp.tile([B, max(TILES) // 4], dt.bfloat16, tag="h2")
                h3 = fp.tile([B, max(TILES) // 8], dt.bfloat16, tag="h3")
                nc.vector.tensor_tensor(out=h1[:, 0:W // 2],
                                        in0=bf[:, 0:W // 2],
                                        in1=bf[:, W // 2:W], op=MAX)
                nc.vector.tensor_tensor(out=h2[:, 0:W // 4],
                                        in0=h1[:, 0:W // 4],
                                        in1=h1[:, W // 4:W // 2], op=MAX)
                nc.vector.tensor_tensor(out=h3[:, 0:W // 8],
                                        in0=h2[:, 0:W // 8],
                                        in1=h2[:, W // 8:W // 4], op=MAX)
                nc.vector.max(vals[:, ti * 8:(ti + 1) * 8], h3[:, 0:W // 8])
                nc.vector.max_index(idxs[:, ti * 8:(ti + 1) * 8],
                                    vals[:, ti * 8:(ti + 1) * 8],
                                    h3[:, 0:W // 8])
            nc.scalar.dma_start(cvals[:], vals[:])
            nc.scalar.dma_start(cidx[:], idxs[:])
    nc.compile()
    _NC_A = nc
    return nc


def _build_b():
    """Program B: beam gather of a hidden slice of state_h+state_c.

    Input st [B, 4*HS]: row b = concat over (state_h l0, state_h l1,
    state_c l0, state_c l1) of beam b's HS-wide hidden slice.  One
    indirect DMA gathers st[q_sel[p]] into partition p; one DMA writes
    newst [B, 4*HS] back out.
    """
    global _NC_B
    if _NC_B is not None:
        return _NC_B
    from concourse import bacc, tile, mybir, bass
    dt = mybir.dt
    HS = H // NCORES

    nc = bacc.Bacc("TRN2", target_bir_lowering=False, debug=False,
                   num_devices=NCORES)
    st = nc.dram_tensor("st", [B, 4 * HS], dt.float32, kind="ExternalInput")
    qidx = nc.dram_tensor("qidx", [B, 1], dt.int32, kind="ExternalInput")
    newst = nc.dram_tensor("newst", [B, 4 * HS], dt.float32,
                           kind="ExternalOutput")

    with tile.TileContext(nc) as tc:
        with tc.tile_pool(name="p", bufs=1) as p:
            qidx_sb = p.tile([B, 1], dt.int32)
            gath = p.tile([B, 4 * HS], dt.float32)
            nc.sync.dma_start(qidx_sb[:], qidx[:])
            nc.gpsimd.indirect_dma_start(
                out=gath[:],
                out_offset=None,
                in_=st[:],
                in_offset=bass.IndirectOffsetOnAxis(ap=qidx_sb[:], axis=0),
            )
            nc.sync.dma_start(newst[:], gath[:])
    nc.compile()
    _NC_B = nc
    return nc


def _ensure_axon_hooks():
    """The agent image's antenv package lacks axon_hooks; shim it so a
    BASS_TRACE=1 environment doesn't crash run_bass_kernel_spmd (tracing
    degrades gracefully when the hook is None)."""
    import sys
    import types
    try:
        import antenv.axon_hooks  # noqa: F401
        return
    except ImportError:
        pass
    mod = types.ModuleType("antenv.axon_hooks")
    holder = [None]
    mod.set_axon_ntff_profile_hook = lambda h: holder.__setitem__(0, h)
    mod.get_axon_ntff_profile_hook = lambda: holder[0]
    sys.modules["antenv.axon_hooks"] = mod
    try:
        import antenv
        antenv.axon_hooks = mod
    except ImportError:
        pass


def _run_spmd(nc, in_maps, tag):
    _ensure_axon_hooks()
    from concourse.bass_utils import run_bass_kernel_spmd
    res = run_bass_kernel_spmd(nc, in_maps, list(range(NCORES)),
                               trace=_TRACE)
    _LAST_RESULTS[tag] = res
    return res.results


def _numpy_reference(logprobsf, beam_seq, beam_seq_logprobs, beam_logprob_sum,
                     state_h, state_c, t, beam_size):
    """Pure-numpy fallback for shapes the device path doesn't cover."""
    lp = np.array(logprobsf, dtype=np.float32)
    Vn = lp.shape[1]
    lp[:, Vn - 1] += -1000.0
    cols = min(beam_size, Vn)
    rows = beam_size if t >= 1 else 1
    part = np.argsort(-lp[:rows], kind="stable", axis=1)[:, :cols]
    ys = np.take_along_axis(lp[:rows], part, axis=1)
    ix = part
    cand_p = beam_logprob_sum[:rows, None] + ys
    flat = cand_p.reshape(-1)
    order = np.argsort(-flat, kind="stable")[:beam_size]
    top_p = flat[order]
    q_sel = order // cols
    c_sel = order % cols
    tok = ix[q_sel, c_sel].astype(beam_seq.dtype)
    loc = ys[q_sel, c_sel]
    new_seq = np.array(beam_seq)
    new_seq[:t] = beam_seq[:t][:, q_sel]
    new_seq[t] = tok
    new_logps = np.array(beam_seq_logprobs)
    new_logps[:t] = beam_seq_logprobs[:t][:, q_sel]
    new_logps[t] = loc
    new_h = state_h[:, q_sel]
    new_c = state_c[:, q_sel]
    return new_seq, new_logps, top_p.astype(np.float32), new_h, new_c


def _select_topk(P_flat, qs, vs, k):
    """Exact top-k of P_flat with jax.lax.top_k tie-breaking ((q, v) asc)."""
    n = P_flat.size
    thr = np.partition(P_flat, n - k)[n - k]
    above = np.nonzero(P_flat > thr)[0]
    need = k - above.size
    if need > 0:
        eq = np.nonzero(P_flat == thr)[0]
        eq = eq[np.lexsort((vs[eq], qs[eq]))][:need]
        sel = np.concatenate([above, eq])
    else:
        sel = above
    order = np.lexsort((vs[sel], qs[sel], -P_flat[sel]))
    return sel[order]


# Window layout: candidate slot s = (tile ti, rank r); pooled index j maps
# to window columns tile_off + j + (W/8)*k, k = 0..7.
_TILE_OFF = np.cumsum((0,) + TILES[:-1])
_TILE_W = np.array(TILES)
_SLOT_TILE = np.repeat(np.arange(NCHUNK), 8)


def _window_cols(core, slot, j):
    """Global vocab columns of a candidate's fold window (8 columns)."""
    ti = _SLOT_TILE[slot]
    w8 = _TILE_W[ti] // 8
    base = core * VSH + _TILE_OFF[ti] + j
    return base + w8 * np.arange(8)


def kernel(logprobsf, beam_seq, beam_seq_logprobs, beam_logprob_sum,
           state_h, state_c, t, beam_size):
    logprobsf = np.asarray(logprobsf, dtype=np.float32)
    beam_seq = np.asarray(beam_seq)
    beam_seq_logprobs = np.asarray(beam_seq_logprobs, dtype=np.float32)
    beam_logprob_sum = np.asarray(beam_logprob_sum, dtype=np.float32)
    state_h = np.asarray(state_h, dtype=np.float32)
    state_c = np.asarray(state_c, dtype=np.float32)
    t = int(t)
    beam_size = int(beam_size)

    if (logprobsf.shape != (B, V) or beam_seq.shape != (T, B)
            or state_h.shape != (L, B, H) or state_c.shape != (L, B, H)
            or beam_size != B or not (0 <= t < T)):
        return _numpy_reference(logprobsf, beam_seq, beam_seq_logprobs,
                                beam_logprob_sum, state_h, state_c, t,
                                beam_size)

    rows = B if t >= 1 else 1

    # ---- Launch A: fold-max candidate scan over vocab shards ----
    ncA = _build_a()
    in_maps = []
    for c in range(NCORES):
        m = {}
        off = c * VSH
        for i, W in enumerate(TILES):
            m[f"lp{i}"] = np.ascontiguousarray(
                logprobsf[:, off:off + W])
            off += W
        biasv = np.zeros((B, 1), np.float32)
        if c == NCORES - 1:
            biasv[:] = -1000.0
        m["bias"] = biasv
        in_maps.append(m)
    resA = _run_spmd(ncA, in_maps, "A")

    cvals = np.stack([np.asarray(resA[c]["cvals"]).astype(np.float32)
                      for c in range(NCORES)], axis=1)
    cidx = np.stack([resA[c]["cidx"].astype(np.int64) for c in range(NCORES)],
                    axis=1)                     # [B, NCORES, CAND]

    # Adjusted logit lookup (EOS column suppressed) for rescans.
    def lp_at(q, v):
        x = logprobsf[q, v].astype(np.float32)
        return np.where(np.asarray(v) == V - 1, x - 1000.0, x).astype(
            np.float32)

    # Candidate totals: global top-128 of bsum[q] + lp[q, v] equals the
    # reference's two-stage top-k.  Candidate values are window maxima;
    # the winners' windows get rescanned exactly on the host below.
    P = beam_logprob_sum[:, None, None] + cvals      # [B, NCORES, CAND]
    if rows < B:
        P[rows:] = -np.inf

    flatP = P.reshape(-1)
    n_all = flatP.size
    qs_all = np.repeat(np.arange(B), NCORES * CAND)
    core_all = np.tile(np.repeat(np.arange(NCORES), CAND), B)
    slot_all = np.tile(np.arange(CAND), B * NCORES)
    j_all = cidx.reshape(-1)

    # Preliminary threshold: 128th largest candidate total, minus a
    # margin covering the bf16 rounding of device candidate values
    # (ulp(v)/2 <= |v| * 2^-9; doubled for safety and floored at the
    # |v| < 16 case that randn logits always hit).
    DELTA = max(0.0625, 2.0 ** -8 * float(np.max(np.abs(cvals))))
    tau = np.partition(flatP, n_all - B)[n_all - B] - 2 * DELTA

    # Validity guard: pooled index in range, no duplicate index inside one
    # chunk's top-8 (HW tie behavior); invalid chunks get a full rescan.
    w8_slot = (_TILE_W[_SLOT_TILE] // 8)
    bad = (j_all < 0) | (j_all >= w8_slot[slot_all])
    idx_g = cidx.reshape(B, NCORES, NCHUNK, 8)
    dup = np.any(np.diff(np.sort(idx_g, axis=-1), axis=-1) == 0, axis=-1)
    bad_chunk = bad.reshape(B, NCORES, NCHUNK, 8).any(axis=-1)
    # Chunks whose smallest reported window-max could still hide an
    # unreported window >= tau.
    v8 = P.reshape(B, NCORES, NCHUNK, 8)[..., 7]
    suspect = (v8 >= tau) | dup | bad_chunk
    if rows < B:
        suspect[rows:] = False

    # Exact rescan pool: full chunks for suspects (expected never), plus
    # the 8-column window of every candidate that can reach tau.
    ex_q, ex_v, ex_p = [], [], []
    seen_chunks = set()
    for q, c, ch in zip(*np.nonzero(suspect)):
        seen_chunks.add((int(q), int(c), int(ch)))
        lo = c * VSH + _TILE_OFF[ch]
        cols = np.arange(lo, lo + _TILE_W[ch])
        ex_q.append(np.full(_TILE_W[ch], q))
        ex_v.append(cols)
        ex_p.append(beam_logprob_sum[q] + lp_at(q, cols))
    need = flatP >= tau
    for i in np.nonzero(need & ~bad)[0]:
        q, c, s, j = qs_all[i], core_all[i], slot_all[i], j_all[i]
        if (int(q), int(c), int(_SLOT_TILE[s])) in seen_chunks:
            continue
        cols = _window_cols(c, s, j)
        ex_q.append(np.full(8, q))
        ex_v.append(cols)
        ex_p.append(beam_logprob_sum[q] + lp_at(q, cols))

    # Every candidate >= tau was rescanned (so the exact pool contains at
    # least 128 entries >= tau); remaining device candidates are < tau and
    # cannot be selected -- drop them and select from the exact pool only.
    flatP2 = np.concatenate(ex_p)
    qs2 = np.concatenate(ex_q)
    vs2 = np.concatenate(ex_v)
    sel = _select_topk(flatP2, qs2, vs2, B)

    top_p = flatP2[sel].astype(np.float32)
    q_sel = qs2[sel].astype(np.int64)
    tok = vs2[sel]
    loc = lp_at(q_sel, tok)

    # ---- Launch B: beam-gather of the recurrent state (hidden-sharded) ----
    ncB = _build_b()
    HS = H // NCORES
    qidx = q_sel[:, None].astype(np.int32)
    in_maps_b = []
    for c in range(NCORES):
        sl = slice(c * HS, (c + 1) * HS)
        stc = np.stack([state_h[0, :, sl], state_h[1, :, sl],
                        state_c[0, :, sl], state_c[1, :, sl]],
                       axis=1).reshape(B, 4 * HS)
        in_maps_b.append({"st": np.ascontiguousarray(stc), "qidx": qidx})
    resB = _run_spmd(ncB, in_maps_b, "B")

    new_h = np.empty_like(state_h)
    new_c = np.empty_like(state_c)
    for c in range(NCORES):
        sl = slice(c * HS, (c + 1) * HS)
        ns = resB[c]["newst"].reshape(B, 4, HS)
        new_h[0, :, sl] = ns[:, 0]
        new_h[1, :, sl] = ns[:, 1]
        new_c[0, :, sl] = ns[:, 2]
        new_c[1, :, sl] = ns[:, 3]

    # ---- Tiny sequence bookkeeping (64 KB int32) ----
    new_seq = np.array(beam_seq)
    new_seq[:t] = beam_seq[:t][:, q_sel]
    new_seq[t] = tok.astype(beam_seq.dtype)
    new_logps = np.array(beam_seq_logprobs)
    new_logps[:t] = beam_seq_logprobs[:t][:, q_sel]
    new_logps[t] = loc

    return new_seq, new_logps, top_p, new_h, new_c


# revision 13
# speedup vs baseline: 1.0621x; 1.0621x over previous
"""Beam-search step (CaptionModel) on 8 Trainium2 NeuronCores.

Distribution strategy (vocab-sharded, per the sharding hint):
  * logprobsf [128, 128000] f32 is sharded along the vocab dim: each of the
    8 cores streams a [128, 16000] shard through SBUF (tile-major
    contiguous DMA), and on the DVE reduces each tile with a fold-max
    cascade (window of 8) before extracting the per-tile top-8
    window-maxima + positions with the Max8 / MaxIndex instructions.
    That's ~1.17 DVE element-passes per input element, roughly matching
    the per-core HBM DMA floor.
  * Key reduction: the reference's per-row top-k followed by the global
    top-k over `beam_logprob_sum[q] + ys` equals one global top-128 of
    `beam_logprob_sum[q] + logprobsf[q, v]`, so the host only merges the
    8 cores' candidate lists (48 window-maxima per row per core, ~100 KB
    total) and rescans the ~128 winning 8-element windows (~1 KB reads).
    An exact threshold check (any chunk whose 8th-largest candidate could
    still hide a winner is rescanned) keeps the result exact, always.
  * A second tiny NEFF gathers the recurrent state along the beam dim with
    one indirect DMA per core (128 row descriptors over a 128-wide hidden
    slice of both states; data-parallel over the hidden dim).

The harness calls kernel(**inputs) with full inputs; both Bass programs are
built/compiled once and cached at module level.
"""

import numpy as np

# Problem geometry (hardcoded per spec).
T = 128          # seq length
B = 128          # beam rows
V = 128000       # vocab
L = 2            # layers
H = 1024         # hidden
NCORES = 8
VSH = V // NCORES                  # 16000 vocab cols per core
WIN = 8                            # fold window
TILES = (1600, 4000, 4000, 4000, 1600, 800)   # sums to VSH
assert sum(TILES) == VSH and all(w % WIN == 0 for w in TILES)
NCHUNK = len(TILES)                # one max8-chunk per tile
CAND = NCHUNK * 8                  # 48 candidates per row per core

_NC_A = None
_NC_B = None
_TRACE = False
_LAST_RESULTS = {}


def _build_a():
    """Program A: per-core fold-max candidate scan of the logit shard."""
    global _NC_A
    if _NC_A is not None:
        return _NC_A
    from concourse import bacc, tile, mybir
    dt = mybir.dt
    MAX = mybir.AluOpType.max

    nc = bacc.Bacc("TRN2", target_bir_lowering=False, debug=False,
                   num_devices=NCORES)
    lps = [nc.dram_tensor(f"lp{i}", [B, W], dt.float32, kind="ExternalInput")
           for i, W in enumerate(TILES)]
    bias = nc.dram_tensor("bias", [B, 1], dt.float32, kind="ExternalInput")
    cvals = nc.dram_tensor("cvals", [B, CAND], dt.bfloat16,
                           kind="ExternalOutput")
    cidx = nc.dram_tensor("cidx", [B, CAND], dt.uint16, kind="ExternalOutput")

    with tile.TileContext(nc) as tc:
        with tc.tile_pool(name="io", bufs=3) as iop, \
             tc.tile_pool(name="fold", bufs=2) as fp, \
             tc.tile_pool(name="cand", bufs=1) as cp:
            vals = cp.tile([B, CAND], dt.bfloat16)
            idxs = cp.tile([B, CAND], dt.uint16)
            bias_t = cp.tile([B, 1], dt.float32)
            # Keep the input (sync/SP) HWDGE ring exclusively for the big
            # tile loads -- it drains FIFO per issuing engine, so anything
            # else on it stalls later tiles.  Small transfers go on the
            # scalar (ACT) ring.
            nc.scalar.dma_start(bias_t[:], bias[:])
            for ti, W in enumerate(TILES):
                t = iop.tile([B, max(TILES)], dt.float32, tag="lp")
                nc.sync.dma_start(t[:, 0:W], lps[ti][:])
                if ti == NCHUNK - 1:
                    # EOS suppression: bias is -1000 on the core owning the
                    # last vocab column, 0 elsewhere (same program on all
                    # cores).
                    nc.vector.tensor_add(out=t[:, W - 1:W],
                                         in0=t[:, W - 1:W], in1=bias_t[:])
                # fold-max cascade: W -> W/2 -> W/4 -> W/8.  Level 1 reads
                # the raw f32 tile and writes bf16 (cast on the write port
                # is free), so levels 2-3 run in the DVE's 2x bf16 mode
                # with no serial ACT cast stage.  The host rescans winner
                # windows against the exact f32 input, so bf16 rounding
                # only widens the rescan margin.
                h1 = fp.tile([B, max(TILES) // 2], dt.bfloat16, tag="h1")
                h2 = fp.tile([B, max(TILES) // 4], dt.bfloat16, tag="h2")
                h3 = fp.tile([B, max(TILES) // 8], dt.bfloat16, tag="h3")
                nc.vector.tensor_tensor(out=h1[:, 0:W // 2],
                                        in0=t[:, 0:W // 2],
                                        in1=t[:, W // 2:W], op=MAX)
                nc.vector.tensor_tensor(out=h2[:, 0:W // 4],
                                        in0=h1[:, 0:W // 4],
                                        in1=h1[:, W // 4:W // 2], op=MAX)
                nc.vector.tensor_tensor(out=h3[:, 0:W // 8],
                                        in0=h2[:, 0:W // 8],
                                        in1=h2[:, W // 8:W // 4], op=MAX)
                nc.vector.max(vals[:, ti * 8:(ti + 1) * 8], h3[:, 0:W // 8])
                nc.vector.max_index(idxs[:, ti * 8:(ti + 1) * 8],
                                    vals[:, ti * 8:(ti + 1) * 8],
                                    h3[:, 0:W // 8])
            nc.scalar.dma_start(cvals[:], vals[:])
            nc.scalar.dma_start(cidx[:], idxs[:])
    nc.compile()
    _NC_A = nc
    return nc


def _build_b():
    """Program B: beam gather of a hidden slice of state_h+state_c.

    Input st [B, 4*HS]: row b = concat over (state_h l0, state_h l1,
    state_c l0, state_c l1) of beam b's HS-wide hidden slice.  One
    indirect DMA gathers st[q_sel[p]] into partition p; one DMA writes
    newst [B, 4*HS] back out.
    """
    global _NC_B
    if _NC_B is not None:
        return _NC_B
    from concourse import bacc, tile, mybir, bass
    dt = mybir.dt
    HS = H // NCORES

    nc = bacc.Bacc("TRN2", target_bir_lowering=False, debug=False,
                   num_devices=NCORES)
    st = nc.dram_tensor("st", [B, 4 * HS], dt.float32, kind="ExternalInput")
    qidx = nc.dram_tensor("qidx", [B, 1], dt.int32, kind="ExternalInput")
    newst = nc.dram_tensor("newst", [B, 4 * HS], dt.float32,
                           kind="ExternalOutput")

    with tile.TileContext(nc) as tc:
        with tc.tile_pool(name="p", bufs=1) as p:
            qidx_sb = p.tile([B, 1], dt.int32)
            gath = p.tile([B, 4 * HS], dt.float32)
            nc.sync.dma_start(qidx_sb[:], qidx[:])
            nc.gpsimd.indirect_dma_start(
                out=gath[:],
                out_offset=None,
                in_=st[:],
                in_offset=bass.IndirectOffsetOnAxis(ap=qidx_sb[:], axis=0),
            )
            nc.sync.dma_start(newst[:], gath[:])
    nc.compile()
    _NC_B = nc
    return nc


def _ensure_axon_hooks():
    """The agent image's antenv package lacks axon_hooks; shim it so a
    BASS_TRACE=1 environment doesn't crash run_bass_kernel_spmd (tracing
    degrades gracefully when the hook is None)."""
    import sys
    import types
    try:
        import antenv.axon_hooks  # noqa: F401
        return
    except ImportError:
        pass
    mod = types.ModuleType("antenv.axon_hooks")
    holder = [None]
    mod.set_axon_ntff_profile_hook = lambda h: holder.__setitem__(0, h)
    mod.get_axon_ntff_profile_hook = lambda: holder[0]
    sys.modules["antenv.axon_hooks"] = mod
    try:
        import antenv
        antenv.axon_hooks = mod
    except ImportError:
        pass


def _run_spmd(nc, in_maps, tag):
    _ensure_axon_hooks()
    from concourse.bass_utils import run_bass_kernel_spmd
    res = run_bass_kernel_spmd(nc, in_maps, list(range(NCORES)),
                               trace=_TRACE)
    _LAST_RESULTS[tag] = res
    return res.results


def _numpy_reference(logprobsf, beam_seq, beam_seq_logprobs, beam_logprob_sum,
                     state_h, state_c, t, beam_size):
    """Pure-numpy fallback for shapes the device path doesn't cover."""
    lp = np.array(logprobsf, dtype=np.float32)
    Vn = lp.shape[1]
    lp[:, Vn - 1] += -1000.0
    cols = min(beam_size, Vn)
    rows = beam_size if t >= 1 else 1
    part = np.argsort(-lp[:rows], kind="stable", axis=1)[:, :cols]
    ys = np.take_along_axis(lp[:rows], part, axis=1)
    ix = part
    cand_p = beam_logprob_sum[:rows, None] + ys
    flat = cand_p.reshape(-1)
    order = np.argsort(-flat, kind="stable")[:beam_size]
    top_p = flat[order]
    q_sel = order // cols
    c_sel = order % cols
    tok = ix[q_sel, c_sel].astype(beam_seq.dtype)
    loc = ys[q_sel, c_sel]
    new_seq = np.array(beam_seq)
    new_seq[:t] = beam_seq[:t][:, q_sel]
    new_seq[t] = tok
    new_logps = np.array(beam_seq_logprobs)
    new_logps[:t] = beam_seq_logprobs[:t][:, q_sel]
    new_logps[t] = loc
    new_h = state_h[:, q_sel]
    new_c = state_c[:, q_sel]
    return new_seq, new_logps, top_p.astype(np.float32), new_h, new_c


def _select_topk(P_flat, qs, vs, k):
    """Exact top-k of P_flat with jax.lax.top_k tie-breaking ((q, v) asc)."""
    n = P_flat.size
    thr = np.partition(P_flat, n - k)[n - k]
    above = np.nonzero(P_flat > thr)[0]
    need = k - above.size
    if need > 0:
        eq = np.nonzero(P_flat == thr)[0]
        eq = eq[np.lexsort((vs[eq], qs[eq]))][:need]
        sel = np.concatenate([above, eq])
    else:
        sel = above
    order = np.lexsort((vs[sel], qs[sel], -P_flat[sel]))
    return sel[order]


# Window layout: candidate slot s = (tile ti, rank r); pooled index j maps
# to window columns tile_off + j + (W/8)*k, k = 0..7.
_TILE_OFF = np.cumsum((0,) + TILES[:-1])
_TILE_W = np.array(TILES)
_SLOT_TILE = np.repeat(np.arange(NCHUNK), 8)


def _window_cols(core, slot, j):
    """Global vocab columns of a candidate's fold window (8 columns)."""
    ti = _SLOT_TILE[slot]
    w8 = _TILE_W[ti] // 8
    base = core * VSH + _TILE_OFF[ti] + j
    return base + w8 * np.arange(8)


def kernel(logprobsf, beam_seq, beam_seq_logprobs, beam_logprob_sum,
           state_h, state_c, t, beam_size):
    logprobsf = np.asarray(logprobsf, dtype=np.float32)
    beam_seq = np.asarray(beam_seq)
    beam_seq_logprobs = np.asarray(beam_seq_logprobs, dtype=np.float32)
    beam_logprob_sum = np.asarray(beam_logprob_sum, dtype=np.float32)
    state_h = np.asarray(state_h, dtype=np.float32)
    state_c = np.asarray(state_c, dtype=np.float32)
    t = int(t)
    beam_size = int(beam_size)

    if (logprobsf.shape != (B, V) or beam_seq.shape != (T, B)
            or state_h.shape != (L, B, H) or state_c.shape != (L, B, H)
            or beam_size != B or not (0 <= t < T)):
        return _numpy_reference(logprobsf, beam_seq, beam_seq_logprobs,
                                beam_logprob_sum, state_h, state_c, t,
                                beam_size)

    rows = B if t >= 1 else 1

    # ---- Launch A: fold-max candidate scan over vocab shards ----
    ncA = _build_a()
    in_maps = []
    for c in range(NCORES):
        m = {}
        off = c * VSH
        for i, W in enumerate(TILES):
            m[f"lp{i}"] = np.ascontiguousarray(
                logprobsf[:, off:off + W])
            off += W
        biasv = np.zeros((B, 1), np.float32)
        if c == NCORES - 1:
            biasv[:] = -1000.0
        m["bias"] = biasv
        in_maps.append(m)
    resA = _run_spmd(ncA, in_maps, "A")

    cvals = np.stack([np.asarray(resA[c]["cvals"]).astype(np.float32)
                      for c in range(NCORES)], axis=1)
    cidx = np.stack([resA[c]["cidx"].astype(np.int64) for c in range(NCORES)],
                    axis=1)                     # [B, NCORES, CAND]

    # Adjusted logit lookup (EOS column suppressed) for rescans.
    def lp_at(q, v):
        x = logprobsf[q, v].astype(np.float32)
        return np.where(np.asarray(v) == V - 1, x - 1000.0, x).astype(
            np.float32)

    # Candidate totals: global top-128 of bsum[q] + lp[q, v] equals the
    # reference's two-stage top-k.  Candidate values are window maxima;
    # the winners' windows get rescanned exactly on the host below.
    P = beam_logprob_sum[:, None, None] + cvals      # [B, NCORES, CAND]
    if rows < B:
        P[rows:] = -np.inf

    flatP = P.reshape(-1)
    n_all = flatP.size
    qs_all = np.repeat(np.arange(B), NCORES * CAND)
    core_all = np.tile(np.repeat(np.arange(NCORES), CAND), B)
    slot_all = np.tile(np.arange(CAND), B * NCORES)
    j_all = cidx.reshape(-1)

    # Preliminary threshold: 128th largest candidate total, minus a
    # margin covering the bf16 rounding of device candidate values
    # (ulp(v)/2 <= |v| * 2^-9; doubled for safety and floored at the
    # |v| < 16 case that randn logits always hit).
    DELTA = max(0.0625, 2.0 ** -8 * float(np.max(np.abs(cvals))))
    tau = np.partition(flatP, n_all - B)[n_all - B] - 2 * DELTA

    # Validity guard: pooled index in range, no duplicate index inside one
    # chunk's top-8 (HW tie behavior); invalid chunks get a full rescan.
    w8_slot = (_TILE_W[_SLOT_TILE] // 8)
    bad = (j_all < 0) | (j_all >= w8_slot[slot_all])
    idx_g = cidx.reshape(B, NCORES, NCHUNK, 8)
    dup = np.any(np.diff(np.sort(idx_g, axis=-1), axis=-1) == 0, axis=-1)
    bad_chunk = bad.reshape(B, NCORES, NCHUNK, 8).any(axis=-1)
    # Chunks whose smallest reported window-max could still hide an
    # unreported window >= tau.
    v8 = P.reshape(B, NCORES, NCHUNK, 8)[..., 7]
    suspect = (v8 >= tau) | dup | bad_chunk
    if rows < B:
        suspect[rows:] = False

    # Exact rescan pool: full chunks for suspects (expected never), plus
    # the 8-column window of every candidate that can reach tau.
    ex_q, ex_v, ex_p = [], [], []
    seen_chunks = set()
    for q, c, ch in zip(*np.nonzero(suspect)):
        seen_chunks.add((int(q), int(c), int(ch)))
        lo = c * VSH + _TILE_OFF[ch]
        cols = np.arange(lo, lo + _TILE_W[ch])
        ex_q.append(np.full(_TILE_W[ch], q))
        ex_v.append(cols)
        ex_p.append(beam_logprob_sum[q] + lp_at(q, cols))
    need = flatP >= tau
    for i in np.nonzero(need & ~bad)[0]:
        q, c, s, j = qs_all[i], core_all[i], slot_all[i], j_all[i]
        if (int(q), int(c), int(_SLOT_TILE[s])) in seen_chunks:
            continue
        cols = _window_cols(c, s, j)
        ex_q.append(np.full(8, q))
        ex_v.append(cols)
        ex_p.append(beam_logprob_sum[q] + lp_at(q, cols))

    # Every candidate >= tau was rescanned (so the exact pool contains at
    # least 128 entries >= tau); remaining device candidates are < tau and
    # cannot be selected -- drop them and select from the exact pool only.
    flatP2 = np.concatenate(ex_p)
    qs2 = np.concatenate(ex_q)
    vs2 = np.concatenate(ex_v)
    sel = _select_topk(flatP2, qs2, vs2, B)

    top_p = flatP2[sel].astype(np.float32)
    q_sel = qs2[sel].astype(np.int64)
    tok = vs2[sel]
    loc = lp_at(q_sel, tok)

    # ---- Launch B: beam-gather of the recurrent state (hidden-sharded) ----
    ncB = _build_b()
    HS = H // NCORES
    qidx = q_sel[:, None].astype(np.int32)
    in_maps_b = []
    for c in range(NCORES):
        sl = slice(c * HS, (c + 1) * HS)
        stc = np.stack([state_h[0, :, sl], state_h[1, :, sl],
                        state_c[0, :, sl], state_c[1, :, sl]],
                       axis=1).reshape(B, 4 * HS)
        in_maps_b.append({"st": np.ascontiguousarray(stc), "qidx": qidx})
    resB = _run_spmd(ncB, in_maps_b, "B")

    new_h = np.empty_like(state_h)
    new_c = np.empty_like(state_c)
    for c in range(NCORES):
        sl = slice(c * HS, (c + 1) * HS)
        ns = resB[c]["newst"].reshape(B, 4, HS)
        new_h[0, :, sl] = ns[:, 0]
        new_h[1, :, sl] = ns[:, 1]
        new_c[0, :, sl] = ns[:, 2]
        new_c[1, :, sl] = ns[:, 3]

    # ---- Tiny sequence bookkeeping (64 KB int32) ----
    new_seq = np.array(beam_seq)
    new_seq[:t] = beam_seq[:t][:, q_sel]
    new_seq[t] = tok.astype(beam_seq.dtype)
    new_logps = np.array(beam_seq_logprobs)
    new_logps[:t] = beam_seq_logprobs[:t][:, q_sel]
    new_logps[t] = loc

    return new_seq, new_logps, top_p, new_h, new_c
